# revision 8
# baseline (speedup 1.0000x reference)
"""CrossFeatureFusion TRN2 kernel.

out[i] = x[i] + sum_{j != i} (x[j] @ W[i,j]^T + b[i,j])
x: [4, 65536, 256] f32, W: [4, 4, 256, 256] f32, b: [4, 4, 256] f32.

Strategy (data-parallel over N, 8 NeuronCores, no collectives):
  - Host: transpose x to feature-major shards xt[core][j, fc, k, n] so the
    contraction dim (f = fc*128 + k) lies on SBUF partitions with no
    on-chip transpose.
  - Host: build block weights M[i][j] = (I if j == i else W[i,j]^T), packed
    per output pair (0,1) / (2,3) as the moving operand.  The identity
    diagonal folds the residual "+ x[i]" into the PSUM accumulation.
  - Device: per 128-row block, 16 fp32r matmuls of [K=128] x [N=512]
    accumulate the full fused output for all 4 modalities in 2 PSUM banks;
    DVE adds the precomputed bias sums while draining PSUM -> SBUF;
    HWDGE DMAs move x-shards in and outputs back.
  - fp32r (TF32-like PE mode) runs at ~1 row/cycle for moving dim >= 256;
    measured rel err vs fp32 reference ~1.5e-4.
"""

import sys

if "/opt/trn_rl_repo" not in sys.path:
    sys.path.insert(0, "/opt/trn_rl_repo")

import numpy as np

M, N, D = 4, 65536, 256
N_CORES = 8
NSH = N // N_CORES  # rows per core
NBLK = NSH // 512  # 512-row blocks per core (v3)
PAIRS = ((0, 1), (2, 3))

_CACHE = {}


def _build_nc(nsh=NSH, repeat=1, xbufs=4, obufs=4, pbufs=4):
    from concourse import bacc
    import concourse.mybir as mybir
    import concourse.tile as tile

    f32 = mybir.dt.float32
    f32r = mybir.dt.float32r
    nblk = nsh // 128

    nc = bacc.Bacc(debug=False)
    xt_d = nc.dram_tensor("xt", [M, 2, 128, nsh], f32r, kind="ExternalInput")
    wp_d = nc.dram_tensor("wp", [2, 8, 128, 512], f32r, kind="ExternalInput")
    bb_d = nc.dram_tensor("bb", [1, 2, 512], f32, kind="ExternalInput")
    out_d = nc.dram_tensor("out", [M, nsh, D], f32, kind="ExternalOutput")

    with tile.TileContext(nc) as tc:
        with (
            tc.tile_pool(name="wsb", bufs=1) as wpool,
            tc.tile_pool(name="xt", bufs=xbufs) as xpool,
            tc.tile_pool(name="osb", bufs=obufs) as opool,
            tc.tile_pool(name="psum", bufs=pbufs, space="PSUM") as ppool,
        ):
            w_sb = wpool.tile([128, 2, 8, 512], f32r)
            nc.sync.dma_start(out=w_sb[:], in_=wp_d.rearrange("p c k e -> k p c e"))
            bias_sb = wpool.tile([128, 2, 512], f32)
            nc.sync.dma_start(
                out=bias_sb[:], in_=bb_d[:].to_broadcast([128, 2, 512])
            )

            def body():
                for nb in range(nblk):
                    n0 = nb * 128
                    xt_sb = xpool.tile([128, M, 2, 128], f32r, name="xt_sb", tag="xt")
                    nc.sync.dma_start(
                        out=xt_sb[:],
                        in_=xt_d[:, :, :, n0 : n0 + 128].rearrange(
                            "j f k n -> k j f n"
                        ),
                    )
                    pss = [
                        ppool.tile([128, 512], f32, tag=f"ps{p}", name=f"ps{p}_{nb}")
                        for p in range(2)
                    ]
                    for c in range(8):
                        j, fc = c >> 1, c & 1
                        for p in range(2):
                            nc.tensor.matmul(
                                pss[p][:],
                                lhsT=xt_sb[:, j, fc, :],
                                rhs=w_sb[:, p, c, :],
                                start=(c == 0),
                                stop=(c == 7),
                            )
                    for p in range(2):
                        o_sb = opool.tile(
                            [128, 2, 256], f32, name=f"osb{p}_{nb}", tag="osb"
                        )
                        nc.vector.tensor_add(
                            out=o_sb[:].rearrange("n i e -> n (i e)"),
                            in0=pss[p][:],
                            in1=bias_sb[:, p, :],
                        )
                        nc.sync.dma_start(
                            out=out_d[2 * p : 2 * p + 2, n0 : n0 + 128, :].rearrange(
                                "i n e -> n i e"
                            ),
                            in_=o_sb[:],
                        )

            if repeat > 1:
                with tc.For_i(0, repeat, 1):
                    body()
            else:
                body()
    nc.finalize()
    return nc


def _build_nc_v2(nsh=NSH, repeat=1, xbufs=3, obufs=6, pbufs=2):
    """out^T formulation: W stationary, xt moving -> PSUM holds out^T[i]
    chunks [128 e, 512 n].  No identity matmuls: the residual "+x[i]" is a
    direct DVE add from the (already transposed) xt tile, fused with the
    bias add in one scalar_tensor_tensor while draining PSUM.  Host
    un-transposes the [4, 2, 128, nsh] output during gather."""
    from concourse import bacc
    import concourse.mybir as mybir
    import concourse.tile as tile

    f32 = mybir.dt.float32
    f32r = mybir.dt.float32r
    NB = 512  # rows per block
    nblk = nsh // NB
    add = mybir.AluOpType.add

    nc = bacc.Bacc(debug=False)
    xt_d = nc.dram_tensor("xt", [M, 2, 128, nsh], f32r, kind="ExternalInput")
    wst_d = nc.dram_tensor("wst", [8, 6, 128, 128], f32r, kind="ExternalInput")
    bbt_d = nc.dram_tensor("bbt", [8, 128], f32, kind="ExternalInput")
    out_d = nc.dram_tensor("out", [M, 2, 128, nsh], f32, kind="ExternalOutput")

    jl = [[j for j in range(M) if j != i] for i in range(M)]

    with tile.TileContext(nc) as tc:
        with (
            tc.tile_pool(name="wsb", bufs=1) as wpool,
            tc.tile_pool(name="xt", bufs=xbufs) as xpool,
            tc.tile_pool(name="osb", bufs=obufs) as opool,
            tc.tile_pool(name="psum", bufs=pbufs, space="PSUM") as ppool,
        ):
            w_sb = wpool.tile([128, 8, 6, 128], f32r)
            nc.sync.dma_start(out=w_sb[:], in_=wst_d.rearrange("t c k m -> k t c m"))
            bias_sb = wpool.tile([128, 8], f32)
            nc.sync.dma_start(out=bias_sb[:], in_=bbt_d.rearrange("t k -> k t"))

            def body():
                for nb in range(nblk):
                    n0 = nb * NB
                    xt_sb = xpool.tile([128, M, 2, NB], f32r, name="xt_sb", tag="xt")
                    nc.sync.dma_start(
                        out=xt_sb[:],
                        in_=xt_d[:, :, :, n0 : n0 + NB].rearrange(
                            "j f k n -> k j f n"
                        ),
                    )
                    for half in range(2):
                        pss = [
                            ppool.tile(
                                [128, NB], f32, tag=f"ps{t}", name=f"ps{t}_{nb}"
                            )
                            for t in range(4)
                        ]
                        for tt in range(4):
                            tg = half * 4 + tt
                            i = tg >> 1
                            for cc in range(6):
                                fc = cc & 1
                                j = jl[i][cc >> 1]
                                nc.tensor.matmul(
                                    pss[tt][:],
                                    lhsT=w_sb[:, tg, cc, :],
                                    rhs=xt_sb[:, j, fc, :],
                                    start=(cc == 0),
                                    stop=(cc == 5),
                                )
                        for tt in range(4):
                            tg = half * 4 + tt
                            i, ec = tg >> 1, tg & 1
                            o_sb = opool.tile(
                                [128, NB], f32, name=f"osb{tg}_{nb}", tag="osb"
                            )
                            nc.vector.scalar_tensor_tensor(
                                out=o_sb[:],
                                in0=pss[tt][:],
                                scalar=bias_sb[:, tg : tg + 1],
                                in1=xt_sb[:, i, ec, :].bitcast(f32),
                                op0=add,
                                op1=add,
                            )
                            nc.sync.dma_start(
                                out=out_d[i, ec, :, n0 : n0 + NB], in_=o_sb[:]
                            )

            if repeat > 1:
                with tc.For_i(0, repeat, 1):
                    body()
            else:
                body()
    nc.finalize()
    return nc


def _build_nc_v3(nsh=NSH, repeat=1, xbufs=3, obufs=3, pbufs=2):
    """bf16 out^T formulation with fully-contiguous DMA layouts.

    Host pre-packs x as xt[k, nb, j, fc, n] bf16 so each block's input DMA
    is one 8KB-contiguous chunk per partition (1MB total); the 8 output
    tiles of a block drain into one [128, 8, NB] f32 SBUF tile and leave in
    a single 16KB-per-partition DMA (2MB).  Weights bf16 -> FWL fast
    weight loads; residual "+x[i]" and bias fold into the PSUM-drain STT.
    """
    from concourse import bacc
    import concourse.mybir as mybir
    import concourse.tile as tile

    f32 = mybir.dt.float32
    bf16 = mybir.dt.bfloat16
    NB = 512
    nblk = nsh // NB
    add = mybir.AluOpType.add

    nc = bacc.Bacc(debug=False)
    xt_d = nc.dram_tensor("xt", [128, nblk, M, 2, NB], bf16, kind="ExternalInput")
    wst_d = nc.dram_tensor("wst", [8, 6, 128, 128], bf16, kind="ExternalInput")
    bbt_d = nc.dram_tensor("bbt", [8, 128], f32, kind="ExternalInput")
    out_d = nc.dram_tensor("out", [128, nblk, 8, NB], f32, kind="ExternalOutput")

    jl = [[j for j in range(M) if j != i] for i in range(M)]

    with tile.TileContext(nc) as tc:
        with (
            tc.tile_pool(name="wsb", bufs=1) as wpool,
            tc.tile_pool(name="xt", bufs=xbufs) as xpool,
            tc.tile_pool(name="osb", bufs=obufs) as opool,
            tc.tile_pool(name="psum", bufs=pbufs, space="PSUM") as ppool,
        ):
            w_sb = wpool.tile([128, 8, 6, 128], bf16)
            nc.sync.dma_start(out=w_sb[:], in_=wst_d.rearrange("t c k m -> k t c m"))
            bias_sb = wpool.tile([128, 8], f32)
            nc.sync.dma_start(out=bias_sb[:], in_=bbt_d.rearrange("t k -> k t"))

            def body():
                for nb in range(nblk):
                    xt_sb = xpool.tile([128, M, 2, NB], bf16, name="xt_sb", tag="xt")
                    nc.sync.dma_start(out=xt_sb[:], in_=xt_d[:, nb])
                    o_sb = opool.tile([128, 8, NB], f32, name=f"osb_{nb}", tag="osb")
                    for half in range(2):
                        pss = [
                            ppool.tile(
                                [128, NB], f32, tag=f"ps{t}", name=f"ps{t}_{nb}"
                            )
                            for t in range(4)
                        ]
                        for tt in range(4):
                            tg = half * 4 + tt
                            i = tg >> 1
                            for cc in range(6):
                                fc = cc & 1
                                j = jl[i][cc >> 1]
                                nc.tensor.matmul(
                                    pss[tt][:],
                                    lhsT=w_sb[:, tg, cc, :],
                                    rhs=xt_sb[:, j, fc, :],
                                    start=(cc == 0),
                                    stop=(cc == 5),
                                )
                        for tt in range(4):
                            tg = half * 4 + tt
                            i, ec = tg >> 1, tg & 1
                            nc.vector.scalar_tensor_tensor(
                                out=o_sb[:, tg, :],
                                in0=pss[tt][:],
                                scalar=bias_sb[:, tg : tg + 1],
                                in1=xt_sb[:, i, ec, :],
                                op0=add,
                                op1=add,
                            )
                    nc.sync.dma_start(out=out_d[:, nb], in_=o_sb[:])

            if repeat > 1:
                with tc.For_i(0, repeat, 1):
                    body()
            else:
                body()
    nc.finalize()
    return nc


def _get_exec(**build_kwargs):
    """Build (once per config) the jitted 8-core executor. Returns a callable
    run(xt_g, wst_g, bbt_g, n_iters) -> out_g with global concat arrays."""
    key = tuple(sorted(build_kwargs.items()))
    if key in _CACHE:
        return _CACHE[key]

    import jax
    import jax.numpy as jnp
    from jax.sharding import Mesh, PartitionSpec
    from jax.experimental.shard_map import shard_map
    from concourse import bass2jax

    nc = _build_nc_v3(**build_kwargs)
    bass2jax.install_neuronx_cc_hook()

    in_names = ["xt", "wst", "bbt", "out"]
    if nc.partition_id_tensor is not None:
        in_names.append(nc.partition_id_tensor.name)
    out_names = ["out"]
    out_aval = jax.core.ShapedArray((128, NBLK, 8, 512), np.float32)

    def _body(xt, wst, bbt, out_zero):
        operands = [xt, wst, bbt, out_zero]
        if nc.partition_id_tensor is not None:
            operands.append(bass2jax.partition_id_tensor())
        outs = bass2jax._bass_exec_p.bind(
            *operands,
            out_avals=(out_aval,),
            in_names=tuple(in_names),
            out_names=tuple(out_names),
            lowering_input_output_aliases=(),
            sim_require_finite=True,
            sim_require_nnan=True,
            nc=nc,
        )
        return tuple(outs)

    devices = jax.devices()[:N_CORES]
    mesh = Mesh(np.asarray(devices), ("core",))
    sharded = jax.jit(
        shard_map(
            _body,
            mesh=mesh,
            in_specs=(PartitionSpec("core"),) * 4,
            out_specs=(PartitionSpec("core"),),
            check_rep=False,
        ),
        keep_unused=True,
    )

    sharding = jax.sharding.NamedSharding(mesh, PartitionSpec("core"))
    _CACHE["sharding"] = sharding
    zeros_fn = jax.jit(
        lambda: jnp.zeros((N_CORES * 128, NBLK, 8, 512), np.float32),
        out_shardings=sharding,
    )

    class Exec:
        def call(self, xt_j, wst_j, bbt_j):
            return sharded(xt_j, wst_j, bbt_j, self.out_buf())[0]

        def out_buf(self):
            if not hasattr(self, "_ob"):
                self._ob = zeros_fn()
                import jax as _jax

                _jax.block_until_ready(self._ob)
            return self._ob

        def run(self, xt_g, wst_g, bbt_g, n_iters=1):
            xt_j = jax.device_put(xt_g, sharding)
            wst_j = jax.device_put(wst_g, sharding)
            bbt_j = jax.device_put(bbt_g, sharding)
            outs = None
            for _ in range(n_iters):
                outs = self.call(xt_j, wst_j, bbt_j)
            jax.block_until_ready(outs)
            return outs

    ex = Exec()
    _CACHE[key] = ex
    return ex


def _prep_inputs(x, W, b):
    """Host-side shard + layout prep. Returns global concatenated arrays."""
    import ml_dtypes

    bf16 = ml_dtypes.bfloat16
    x = np.asarray(x, dtype=np.float32)
    W = np.asarray(W, dtype=np.float32)
    b = np.asarray(b, dtype=np.float32)
    n = x.shape[1]
    nsh = n // N_CORES
    nblk = nsh // 512

    # xt_g[c*128 + k, nb, j, fc, n] = x[j, c*nsh + nb*512 + n, fc*128 + k]
    x6 = x.astype(bf16).reshape(M, N_CORES, nblk, 512, 2, 128)
    xt_g = np.ascontiguousarray(x6.transpose(1, 5, 2, 0, 4, 3)).reshape(
        N_CORES * 128, nblk, M, 2, 512
    )

    # Stationary W chunks: wst[(i*2+ec), cc, k, m] = W[i, jl[cc>>1]].T block
    wst = np.empty((8, 6, 128, 128), dtype=np.float32)
    for i in range(M):
        jli = [j for j in range(M) if j != i]
        for ec in range(2):
            t = i * 2 + ec
            for cc in range(6):
                j = jli[cc >> 1]
                fc = cc & 1
                wst[t, cc] = W[i, j][
                    ec * 128 : (ec + 1) * 128, fc * 128 : (fc + 1) * 128
                ].T
    wst_g = np.ascontiguousarray(
        np.broadcast_to(wst.astype(bf16)[None], (N_CORES, 8, 6, 128, 128))
    ).reshape(N_CORES * 8, 6, 128, 128)

    # bias sums: BS[i] = sum_{j != i} b[i, j];  bbt[(i*2+ec), k]
    bs = b.sum(axis=1) - b[np.arange(M), np.arange(M)]  # [4, 256]
    bbt = bs.reshape(8, 128)
    bbt_g = np.ascontiguousarray(
        np.broadcast_to(bbt[None], (N_CORES, 8, 128))
    ).reshape(N_CORES * 8, 128)

    return xt_g, wst_g, bbt_g


def kernel(x, W, b):
    xt_g, wst_g, bbt_g = _prep_inputs(x, W, b)
    ex = _get_exec()
    out_g = ex.run(xt_g, wst_g, bbt_g)
    # out_g: [NC*128, NBLK, 8, 512]; out[i, c*NSH+nb*512+n, ec*128+e]
    #   = out_g[c*128+e, nb, i*2+ec, n]
    og = np.asarray(out_g).reshape(N_CORES, 128, NBLK, M, 2, 512)
    out = np.ascontiguousarray(og.transpose(3, 0, 2, 5, 4, 1)).reshape(M, N, D)
    return out



# revision 34
# speedup vs baseline: 4.0510x; 4.0510x over previous
"""CrossFeatureFusion TRN2 kernel.

out[i] = x[i] + sum_{j != i} (x[j] @ W[i,j]^T + b[i,j])
x: [4, 65536, 256] f32, W: [4, 4, 256, 256] f32, b: [4, 4, 256] f32.

Strategy (data-parallel over N, 8 NeuronCores, no collectives):
  - Host: transpose x to feature-major shards xt[core][j, fc, k, n] so the
    contraction dim (f = fc*128 + k) lies on SBUF partitions with no
    on-chip transpose.
  - Host: build block weights M[i][j] = (I if j == i else W[i,j]^T), packed
    per output pair (0,1) / (2,3) as the moving operand.  The identity
    diagonal folds the residual "+ x[i]" into the PSUM accumulation.
  - Device: per 128-row block, 16 fp32r matmuls of [K=128] x [N=512]
    accumulate the full fused output for all 4 modalities in 2 PSUM banks;
    DVE adds the precomputed bias sums while draining PSUM -> SBUF;
    HWDGE DMAs move x-shards in and outputs back.
  - fp32r (TF32-like PE mode) runs at ~1 row/cycle for moving dim >= 256;
    measured rel err vs fp32 reference ~1.5e-4.
"""

import sys

if "/opt/trn_rl_repo" not in sys.path:
    sys.path.insert(0, "/opt/trn_rl_repo")

import numpy as np

M, N, D = 4, 65536, 256
N_CORES = 8
NSH = N // N_CORES  # rows per core
NBLK = NSH // 512  # 512-row blocks per core (v3)
PAIRS = ((0, 1), (2, 3))

_CACHE = {}


def _build_nc(nsh=NSH, repeat=1, xbufs=4, obufs=4, pbufs=4):
    from concourse import bacc
    import concourse.mybir as mybir
    import concourse.tile as tile

    f32 = mybir.dt.float32
    f32r = mybir.dt.float32r
    nblk = nsh // 128

    nc = bacc.Bacc(debug=False)
    xt_d = nc.dram_tensor("xt", [M, 2, 128, nsh], f32r, kind="ExternalInput")
    wp_d = nc.dram_tensor("wp", [2, 8, 128, 512], f32r, kind="ExternalInput")
    bb_d = nc.dram_tensor("bb", [1, 2, 512], f32, kind="ExternalInput")
    out_d = nc.dram_tensor("out", [M, nsh, D], f32, kind="ExternalOutput")

    with tile.TileContext(nc) as tc:
        with (
            tc.tile_pool(name="wsb", bufs=1) as wpool,
            tc.tile_pool(name="xt", bufs=xbufs) as xpool,
            tc.tile_pool(name="osb", bufs=obufs) as opool,
            tc.tile_pool(name="psum", bufs=pbufs, space="PSUM") as ppool,
        ):
            w_sb = wpool.tile([128, 2, 8, 512], f32r)
            nc.sync.dma_start(out=w_sb[:], in_=wp_d.rearrange("p c k e -> k p c e"))
            bias_sb = wpool.tile([128, 2, 512], f32)
            nc.sync.dma_start(
                out=bias_sb[:], in_=bb_d[:].to_broadcast([128, 2, 512])
            )

            def body():
                for nb in range(nblk):
                    n0 = nb * 128
                    xt_sb = xpool.tile([128, M, 2, 128], f32r, name="xt_sb", tag="xt")
                    nc.sync.dma_start(
                        out=xt_sb[:],
                        in_=xt_d[:, :, :, n0 : n0 + 128].rearrange(
                            "j f k n -> k j f n"
                        ),
                    )
                    pss = [
                        ppool.tile([128, 512], f32, tag=f"ps{p}", name=f"ps{p}_{nb}")
                        for p in range(2)
                    ]
                    for c in range(8):
                        j, fc = c >> 1, c & 1
                        for p in range(2):
                            nc.tensor.matmul(
                                pss[p][:],
                                lhsT=xt_sb[:, j, fc, :],
                                rhs=w_sb[:, p, c, :],
                                start=(c == 0),
                                stop=(c == 7),
                            )
                    for p in range(2):
                        o_sb = opool.tile(
                            [128, 2, 256], f32, name=f"osb{p}_{nb}", tag="osb"
                        )
                        nc.vector.tensor_add(
                            out=o_sb[:].rearrange("n i e -> n (i e)"),
                            in0=pss[p][:],
                            in1=bias_sb[:, p, :],
                        )
                        nc.sync.dma_start(
                            out=out_d[2 * p : 2 * p + 2, n0 : n0 + 128, :].rearrange(
                                "i n e -> n i e"
                            ),
                            in_=o_sb[:],
                        )

            if repeat > 1:
                with tc.For_i(0, repeat, 1):
                    body()
            else:
                body()
    nc.finalize()
    return nc


def _build_nc_v2(nsh=NSH, repeat=1, xbufs=3, obufs=6, pbufs=2):
    """out^T formulation: W stationary, xt moving -> PSUM holds out^T[i]
    chunks [128 e, 512 n].  No identity matmuls: the residual "+x[i]" is a
    direct DVE add from the (already transposed) xt tile, fused with the
    bias add in one scalar_tensor_tensor while draining PSUM.  Host
    un-transposes the [4, 2, 128, nsh] output during gather."""
    from concourse import bacc
    import concourse.mybir as mybir
    import concourse.tile as tile

    f32 = mybir.dt.float32
    f32r = mybir.dt.float32r
    NB = 512  # rows per block
    nblk = nsh // NB
    add = mybir.AluOpType.add

    nc = bacc.Bacc(debug=False)
    xt_d = nc.dram_tensor("xt", [M, 2, 128, nsh], f32r, kind="ExternalInput")
    wst_d = nc.dram_tensor("wst", [8, 6, 128, 128], f32r, kind="ExternalInput")
    bbt_d = nc.dram_tensor("bbt", [8, 128], f32, kind="ExternalInput")
    out_d = nc.dram_tensor("out", [M, 2, 128, nsh], f32, kind="ExternalOutput")

    jl = [[j for j in range(M) if j != i] for i in range(M)]

    with tile.TileContext(nc) as tc:
        with (
            tc.tile_pool(name="wsb", bufs=1) as wpool,
            tc.tile_pool(name="xt", bufs=xbufs) as xpool,
            tc.tile_pool(name="osb", bufs=obufs) as opool,
            tc.tile_pool(name="psum", bufs=pbufs, space="PSUM") as ppool,
        ):
            w_sb = wpool.tile([128, 8, 6, 128], f32r)
            nc.sync.dma_start(out=w_sb[:], in_=wst_d.rearrange("t c k m -> k t c m"))
            bias_sb = wpool.tile([128, 8], f32)
            nc.sync.dma_start(out=bias_sb[:], in_=bbt_d.rearrange("t k -> k t"))

            def body():
                for nb in range(nblk):
                    n0 = nb * NB
                    xt_sb = xpool.tile([128, M, 2, NB], f32r, name="xt_sb", tag="xt")
                    nc.sync.dma_start(
                        out=xt_sb[:],
                        in_=xt_d[:, :, :, n0 : n0 + NB].rearrange(
                            "j f k n -> k j f n"
                        ),
                    )
                    for half in range(2):
                        pss = [
                            ppool.tile(
                                [128, NB], f32, tag=f"ps{t}", name=f"ps{t}_{nb}"
                            )
                            for t in range(4)
                        ]
                        for tt in range(4):
                            tg = half * 4 + tt
                            i = tg >> 1
                            for cc in range(6):
                                fc = cc & 1
                                j = jl[i][cc >> 1]
                                nc.tensor.matmul(
                                    pss[tt][:],
                                    lhsT=w_sb[:, tg, cc, :],
                                    rhs=xt_sb[:, j, fc, :],
                                    start=(cc == 0),
                                    stop=(cc == 5),
                                )
                        for tt in range(4):
                            tg = half * 4 + tt
                            i, ec = tg >> 1, tg & 1
                            o_sb = opool.tile(
                                [128, NB], f32, name=f"osb{tg}_{nb}", tag="osb"
                            )
                            nc.vector.scalar_tensor_tensor(
                                out=o_sb[:],
                                in0=pss[tt][:],
                                scalar=bias_sb[:, tg : tg + 1],
                                in1=xt_sb[:, i, ec, :].bitcast(f32),
                                op0=add,
                                op1=add,
                            )
                            nc.sync.dma_start(
                                out=out_d[i, ec, :, n0 : n0 + NB], in_=o_sb[:]
                            )

            if repeat > 1:
                with tc.For_i(0, repeat, 1):
                    body()
            else:
                body()
    nc.finalize()
    return nc


def _build_nc_v3(
    nsh=NSH,
    repeat=1,
    xbufs=3,
    obufs=3,
    pbufs=2,
    mode="full",
    stagger=0,
    hints=1,
    dblk=1,
    split=0,
    hiprio=0,
    obf=1,
    pair=0,
    ldwskip=0,
    ccouter=0,
    mmsplit=1,
):
    """bf16 out^T formulation with fully-contiguous DMA layouts.

    Host pre-packs x as xt[k, nb, j, fc, n] bf16 so each block's input DMA
    is one 8KB-contiguous chunk per partition (1MB total); the 8 output
    tiles of a block drain into one [128, 8, NB] f32 SBUF tile and leave in
    a single 16KB-per-partition DMA (2MB).  Weights bf16 -> FWL fast
    weight loads; residual "+x[i]" and bias fold into the PSUM-drain STT.
    """
    from concourse import bacc
    import concourse.mybir as mybir
    import concourse.tile as tile

    f32 = mybir.dt.float32
    bf16 = mybir.dt.bfloat16
    NB = 512
    nblk = nsh // NB
    add = mybir.AluOpType.add

    odt = bf16 if obf else f32
    nc = bacc.Bacc(debug=False)
    xt_d = nc.dram_tensor("xt", [128, nblk, M, 2, NB], bf16, kind="ExternalInput")
    wst_d = nc.dram_tensor("wst", [8, 6, 128, 128], bf16, kind="ExternalInput")
    bbt_d = nc.dram_tensor("bbt", [8, 128], f32, kind="ExternalInput")
    out_d = nc.dram_tensor("out", [128, nblk, 8, NB], odt, kind="ExternalOutput")

    jl = [[j for j in range(M) if j != i] for i in range(M)]

    with tile.TileContext(nc) as tc:
        with (
            tc.tile_pool(name="wsb", bufs=1) as wpool,
            tc.tile_pool(name="xt", bufs=xbufs) as xpool,
            tc.tile_pool(name="osb", bufs=obufs) as opool,
            tc.tile_pool(name="psum", bufs=pbufs, space="PSUM") as ppool,
        ):
            w_sb = wpool.tile([128, 8, 6, 128], bf16)
            nc.sync.dma_start(out=w_sb[:], in_=wst_d.rearrange("t c k m -> k t c m"))
            bias_sb = wpool.tile([128, 8], f32)
            nc.sync.dma_start(out=bias_sb[:], in_=bbt_d.rearrange("t k -> k t"))
            if mode == "peonly":
                xt_fix = wpool.tile([128, M, 2, NB], bf16)
                nc.sync.dma_start(out=xt_fix[:], in_=xt_d[:, 0])

            def compute_block(nb, xt_b, o_sb):
                # xt_b: [128, M, 2, NB] view; o_sb: [128, 8, NB] tile
                for half in range(2):
                    pss = [
                        ppool.tile([128, NB], f32, tag=f"ps{t}", name=f"ps{t}_{nb}")
                        for t in range(4)
                    ]
                    ncc = 3 if mode == "halfmm" else 6
                    if ccouter:
                        order = [(tt, cc) for cc in range(ncc) for tt in range(4)]
                    else:
                        order = [(tt, cc) for tt in range(4) for cc in range(ncc)]
                    nbs = NB // mmsplit
                    for tt, cc in order:
                        tg = half * 4 + tt
                        i = tg >> 1
                        fc = cc & 1
                        j = jl[i][cc >> 1]
                        for s in range(mmsplit):
                            nc.tensor.matmul(
                                pss[tt][:, s * nbs : (s + 1) * nbs],
                                lhsT=w_sb[:, tg, cc, :],
                                rhs=xt_b[:, j, fc, s * nbs : (s + 1) * nbs],
                                start=(cc == 0),
                                stop=(cc == ncc - 1),
                            )
                    for tt in range(4):
                        tg = half * 4 + tt
                        i, ec = tg >> 1, tg & 1
                        nc.vector.scalar_tensor_tensor(
                            out=o_sb[:, tg, :],
                            in0=pss[tt][:],
                            scalar=bias_sb[:, tg : tg + 1],
                            in1=xt_b[:, i, ec, :],
                            op0=add,
                            op1=add,
                        )
                    if split and half == 0:
                        nc.scalar.dma_start(
                            out=out_d[:, nb, 0:4], in_=o_sb[:, 0:4, :]
                        )
                if mode == "peonly":
                    nc.scalar.dma_start(out=out_d[:, nb, :, 0:8], in_=o_sb[:, :, 0:8])
                elif split:
                    nc.scalar.dma_start(out=out_d[:, nb, 4:8], in_=o_sb[:, 4:8, :])
                else:
                    nc.scalar.dma_start(out=out_d[:, nb], in_=o_sb[:])

            def compute_pair(nb0, xt_sb, o_sbs):
                # xt_sb: [128, 2, M, 2, NB]; o_sbs: two [128, 8, NB] tiles.
                # Each stationary weight is loaded once and used for both
                # blocks (ldwskip=1 sets ldweights=False on the 2nd matmul).
                for half in range(2):
                    pss = [
                        [
                            ppool.tile(
                                [128, NB],
                                f32,
                                tag=f"ps{t}b{bb}",
                                name=f"ps{t}b{bb}_{nb0}",
                            )
                            for bb in range(2)
                        ]
                        for t in range(4)
                    ]
                    for tt in range(4):
                        tg = half * 4 + tt
                        i = tg >> 1
                        for cc in range(6):
                            fc = cc & 1
                            j = jl[i][cc >> 1]
                            for bb in range(2):
                                mm = nc.tensor.matmul(
                                    pss[tt][bb][:],
                                    lhsT=w_sb[:, tg, cc, :],
                                    rhs=xt_sb[:, bb, j, fc, :],
                                    start=(cc == 0),
                                    stop=(cc == 5),
                                )
                                if ldwskip and bb == 1:
                                    mm.ins.ldweights = False
                    for tt in range(4):
                        tg = half * 4 + tt
                        i, ec = tg >> 1, tg & 1
                        for bb in range(2):
                            nc.vector.scalar_tensor_tensor(
                                out=o_sbs[bb][:, tg, :],
                                in0=pss[tt][bb][:],
                                scalar=bias_sb[:, tg : tg + 1],
                                in1=xt_sb[:, bb, i, ec, :],
                                op0=add,
                                op1=add,
                            )

            def body_pair():
                for nb0 in range(0, nblk, 2):
                    xt_sb = xpool.tile([128, 2, M, 2, NB], bf16, name="xt_sb", tag="xt")
                    nc.sync.dma_start(out=xt_sb[:], in_=xt_d[:, nb0 : nb0 + 2])
                    o_sbs = [
                        opool.tile(
                            [128, 8, NB], odt, name=f"osb_{nb0 + bb}", tag="osb"
                        )
                        for bb in range(2)
                    ]
                    compute_pair(nb0, xt_sb, o_sbs)
                    for bb in range(2):
                        nc.scalar.dma_start(out=out_d[:, nb0 + bb], in_=o_sbs[bb][:])

            def body_peonly():
                for nb in range(nblk):
                    o_sb = opool.tile([128, 8, NB], odt, name=f"osb_{nb}", tag="osb")
                    compute_block(nb, xt_fix, o_sb)

            def body():
                if mode == "peonly":
                    body_peonly()
                    return
                if pair:
                    body_pair()
                    return
                for nb0 in range(0, nblk, dblk):
                    xt_sb = xpool.tile(
                        [128, dblk, M, 2, NB], bf16, name="xt_sb", tag="xt"
                    )
                    if mode != "noin":
                        if hiprio:
                            with tc.high_priority():
                                nc.sync.dma_start(
                                    out=xt_sb[:], in_=xt_d[:, nb0 : nb0 + dblk]
                                )
                        else:
                            nc.sync.dma_start(
                                out=xt_sb[:], in_=xt_d[:, nb0 : nb0 + dblk]
                            )
                    for db in range(dblk):
                        nb = nb0 + db
                        o_sb = opool.tile(
                            [128, 8, NB], odt, name=f"osb_{nb}", tag="osb"
                        )
                        if mode == "dma":
                            nc.gpsimd.memset(o_sb[:], 0.0)
                            nc.scalar.dma_start(out=out_d[:, nb], in_=o_sb[:])
                            continue
                        if mode == "noout":
                            compute_block(nb, xt_sb[:, db], o_sb)  # type: ignore
                            continue
                        compute_block(nb, xt_sb[:, db], o_sb)

            if repeat > 1:
                kw = {}
                if stagger:
                    kw["staggered_reset"] = True
                if hints:
                    kw["hint_engines"] = (mybir.EngineType.PE,)
                with tc.For_i(0, repeat, 1, **kw):
                    body()
            else:
                body()
    nc.finalize()
    return nc


def _get_exec(**build_kwargs):
    """Build (once per config) the jitted 8-core executor. Returns a callable
    run(xt_g, wst_g, bbt_g, n_iters) -> out_g with global concat arrays."""
    build_kwargs = {"obf": 1, "hints": 1, **build_kwargs}
    key = tuple(sorted(build_kwargs.items()))
    if key in _CACHE:
        return _CACHE[key]

    import jax
    import jax.numpy as jnp
    from jax.sharding import Mesh, PartitionSpec
    from jax.experimental.shard_map import shard_map
    from concourse import bass2jax

    nc = _build_nc_v3(**build_kwargs)
    bass2jax.install_neuronx_cc_hook()

    in_names = ["xt", "wst", "bbt", "out"]
    if nc.partition_id_tensor is not None:
        in_names.append(nc.partition_id_tensor.name)
    out_names = ["out"]
    import ml_dtypes

    out_np_dt = ml_dtypes.bfloat16 if build_kwargs.get("obf") else np.float32
    out_aval = jax.core.ShapedArray((128, NBLK, 8, 512), out_np_dt)

    def _body(xt, wst, bbt, out_zero):
        operands = [xt, wst, bbt, out_zero]
        if nc.partition_id_tensor is not None:
            operands.append(bass2jax.partition_id_tensor())
        outs = bass2jax._bass_exec_p.bind(
            *operands,
            out_avals=(out_aval,),
            in_names=tuple(in_names),
            out_names=tuple(out_names),
            lowering_input_output_aliases=(),
            sim_require_finite=True,
            sim_require_nnan=True,
            nc=nc,
        )
        return tuple(outs)

    devices = jax.devices()[:N_CORES]
    mesh = Mesh(np.asarray(devices), ("core",))
    sharded = jax.jit(
        shard_map(
            _body,
            mesh=mesh,
            in_specs=(PartitionSpec("core"),) * 4,
            out_specs=(PartitionSpec("core"),),
            check_rep=False,
        ),
        keep_unused=True,
    )

    sharding = jax.sharding.NamedSharding(mesh, PartitionSpec("core"))
    _CACHE["sharding"] = sharding
    zeros_fn = jax.jit(
        lambda: jnp.zeros((N_CORES * 128, NBLK, 8, 512), out_np_dt),
        out_shardings=sharding,
    )

    class Exec:
        def call(self, xt_j, wst_j, bbt_j):
            return sharded(xt_j, wst_j, bbt_j, self.out_buf())[0]

        def out_buf(self):
            if not hasattr(self, "_ob"):
                self._ob = zeros_fn()
                import jax as _jax

                _jax.block_until_ready(self._ob)
            return self._ob

        def run(self, xt_g, wst_g, bbt_g, n_iters=1):
            xt_j = jax.device_put(xt_g, sharding)
            wst_j = jax.device_put(wst_g, sharding)
            bbt_j = jax.device_put(bbt_g, sharding)
            outs = None
            for _ in range(n_iters):
                outs = self.call(xt_j, wst_j, bbt_j)
            jax.block_until_ready(outs)
            return outs

    ex = Exec()
    _CACHE[key] = ex
    return ex


def _prep_inputs(x, W, b):
    """Host-side shard + layout prep. Returns global concatenated arrays."""
    import ml_dtypes

    bf16 = ml_dtypes.bfloat16
    x = np.asarray(x, dtype=np.float32)
    W = np.asarray(W, dtype=np.float32)
    b = np.asarray(b, dtype=np.float32)
    n = x.shape[1]
    nsh = n // N_CORES
    nblk = nsh // 512

    # xt_g[c*128 + k, nb, j, fc, n] = x[j, c*nsh + nb*512 + n, fc*128 + k]
    x6 = x.astype(bf16).reshape(M, N_CORES, nblk, 512, 2, 128)
    xt_g = np.ascontiguousarray(x6.transpose(1, 5, 2, 0, 4, 3)).reshape(
        N_CORES * 128, nblk, M, 2, 512
    )

    # Stationary W chunks: wst[(i*2+ec), cc, k, m] = W[i, jl[cc>>1]].T block
    wst = np.empty((8, 6, 128, 128), dtype=np.float32)
    for i in range(M):
        jli = [j for j in range(M) if j != i]
        for ec in range(2):
            t = i * 2 + ec
            for cc in range(6):
                j = jli[cc >> 1]
                fc = cc & 1
                wst[t, cc] = W[i, j][
                    ec * 128 : (ec + 1) * 128, fc * 128 : (fc + 1) * 128
                ].T
    wst_g = np.ascontiguousarray(
        np.broadcast_to(wst.astype(bf16)[None], (N_CORES, 8, 6, 128, 128))
    ).reshape(N_CORES * 8, 6, 128, 128)

    # bias sums: BS[i] = sum_{j != i} b[i, j];  bbt[(i*2+ec), k]
    bs = b.sum(axis=1) - b[np.arange(M), np.arange(M)]  # [4, 256]
    bbt = bs.reshape(8, 128)
    bbt_g = np.ascontiguousarray(
        np.broadcast_to(bbt[None], (N_CORES, 8, 128))
    ).reshape(N_CORES * 8, 128)

    return xt_g, wst_g, bbt_g


def kernel(x, W, b):
    xt_g, wst_g, bbt_g = _prep_inputs(x, W, b)
    ex = _get_exec()
    out_g = ex.run(xt_g, wst_g, bbt_g)
    # out_g: [NC*128, NBLK, 8, 512]; out[i, c*NSH+nb*512+n, ec*128+e]
    #   = out_g[c*128+e, nb, i*2+ec, n]
    og = np.asarray(out_g).reshape(N_CORES, 128, NBLK, M, 2, 512)
    out = np.ascontiguousarray(og.transpose(3, 0, 2, 5, 4, 1))
    if out.dtype != np.float32:
        out = out.astype(np.float32)
    out = out.reshape(M, N, D)
    return out



# revision 39
# speedup vs baseline: 4.3053x; 1.0628x over previous
"""CrossFeatureFusion TRN2 kernel.

out[i] = x[i] + sum_{j != i} (x[j] @ W[i,j]^T + b[i,j])
x: [4, 65536, 256] f32, W: [4, 4, 256, 256] f32, b: [4, 4, 256] f32.

Strategy (data-parallel over N, 8 NeuronCores, no collectives) — v3:
  - Host: cast x/W to bf16 and pre-pack x as xt[core][k, nb, j, fc, n] so
    every 512-row block's input DMA is one 8KB-contiguous chunk per SBUF
    partition (1MB per DMA).  bf16 matmul rel-err ~2e-3 << the 2e-2 gate.
  - Device (out^T formulation): W blocks stationary, x moving; PSUM holds
    out^T chunks [128 e, 512 n].  Per block 48 bf16 matmuls (4 targets x 2
    halves x 6 K-chunks) accumulate in 8 PSUM banks (4 tags double-
    buffered); DVE scalar_tensor_tensor drains PSUM fusing bias + the
    "+x[i]" residual and writes a bf16 [128, 8, 512] output tile; host
    upcasts to f32 on gather (adds ~2e-3 rounding, total rel err ~2.6e-3).
  - Input DMAs issue on nc.sync (SP); output DMAs on nc.scalar (ACT).
    Splitting the two HWDGE streams matters a LOT: both on SP serializes
    input prefetch behind the output DMA's wait-for-compute (871us -> 205us).
  - Measured: ~196-215us per full workload (8 cores), vs ~197us PE floor
    (768 matmuls x 512 cols at the ~2.0GHz sustained-throttled PE clock;
    DMA with bf16 in/out is ~100us, fully hidden).
"""

import sys

if "/opt/trn_rl_repo" not in sys.path:
    sys.path.insert(0, "/opt/trn_rl_repo")

import numpy as np

M, N, D = 4, 65536, 256
N_CORES = 8
NSH = N // N_CORES  # rows per core
NBLK = NSH // 512  # 512-row blocks per core (v3)
PAIRS = ((0, 1), (2, 3))

_CACHE = {}


def _build_nc(nsh=NSH, repeat=1, xbufs=4, obufs=4, pbufs=4):
    from concourse import bacc
    import concourse.mybir as mybir
    import concourse.tile as tile

    f32 = mybir.dt.float32
    f32r = mybir.dt.float32r
    nblk = nsh // 128

    nc = bacc.Bacc(debug=False)
    xt_d = nc.dram_tensor("xt", [M, 2, 128, nsh], f32r, kind="ExternalInput")
    wp_d = nc.dram_tensor("wp", [2, 8, 128, 512], f32r, kind="ExternalInput")
    bb_d = nc.dram_tensor("bb", [1, 2, 512], f32, kind="ExternalInput")
    out_d = nc.dram_tensor("out", [M, nsh, D], f32, kind="ExternalOutput")

    with tile.TileContext(nc) as tc:
        with (
            tc.tile_pool(name="wsb", bufs=1) as wpool,
            tc.tile_pool(name="xt", bufs=xbufs) as xpool,
            tc.tile_pool(name="osb", bufs=obufs) as opool,
            tc.tile_pool(name="psum", bufs=pbufs, space="PSUM") as ppool,
        ):
            w_sb = wpool.tile([128, 2, 8, 512], f32r)
            nc.sync.dma_start(out=w_sb[:], in_=wp_d.rearrange("p c k e -> k p c e"))
            bias_sb = wpool.tile([128, 2, 512], f32)
            nc.sync.dma_start(
                out=bias_sb[:], in_=bb_d[:].to_broadcast([128, 2, 512])
            )

            def body():
                for nb in range(nblk):
                    n0 = nb * 128
                    xt_sb = xpool.tile([128, M, 2, 128], f32r, name="xt_sb", tag="xt")
                    nc.sync.dma_start(
                        out=xt_sb[:],
                        in_=xt_d[:, :, :, n0 : n0 + 128].rearrange(
                            "j f k n -> k j f n"
                        ),
                    )
                    pss = [
                        ppool.tile([128, 512], f32, tag=f"ps{p}", name=f"ps{p}_{nb}")
                        for p in range(2)
                    ]
                    for c in range(8):
                        j, fc = c >> 1, c & 1
                        for p in range(2):
                            nc.tensor.matmul(
                                pss[p][:],
                                lhsT=xt_sb[:, j, fc, :],
                                rhs=w_sb[:, p, c, :],
                                start=(c == 0),
                                stop=(c == 7),
                            )
                    for p in range(2):
                        o_sb = opool.tile(
                            [128, 2, 256], f32, name=f"osb{p}_{nb}", tag="osb"
                        )
                        nc.vector.tensor_add(
                            out=o_sb[:].rearrange("n i e -> n (i e)"),
                            in0=pss[p][:],
                            in1=bias_sb[:, p, :],
                        )
                        nc.sync.dma_start(
                            out=out_d[2 * p : 2 * p + 2, n0 : n0 + 128, :].rearrange(
                                "i n e -> n i e"
                            ),
                            in_=o_sb[:],
                        )

            if repeat > 1:
                with tc.For_i(0, repeat, 1):
                    body()
            else:
                body()
    nc.finalize()
    return nc


def _build_nc_v2(nsh=NSH, repeat=1, xbufs=3, obufs=6, pbufs=2):
    """out^T formulation: W stationary, xt moving -> PSUM holds out^T[i]
    chunks [128 e, 512 n].  No identity matmuls: the residual "+x[i]" is a
    direct DVE add from the (already transposed) xt tile, fused with the
    bias add in one scalar_tensor_tensor while draining PSUM.  Host
    un-transposes the [4, 2, 128, nsh] output during gather."""
    from concourse import bacc
    import concourse.mybir as mybir
    import concourse.tile as tile

    f32 = mybir.dt.float32
    f32r = mybir.dt.float32r
    NB = 512  # rows per block
    nblk = nsh // NB
    add = mybir.AluOpType.add

    nc = bacc.Bacc(debug=False)
    xt_d = nc.dram_tensor("xt", [M, 2, 128, nsh], f32r, kind="ExternalInput")
    wst_d = nc.dram_tensor("wst", [8, 6, 128, 128], f32r, kind="ExternalInput")
    bbt_d = nc.dram_tensor("bbt", [8, 128], f32, kind="ExternalInput")
    out_d = nc.dram_tensor("out", [M, 2, 128, nsh], f32, kind="ExternalOutput")

    jl = [[j for j in range(M) if j != i] for i in range(M)]

    with tile.TileContext(nc) as tc:
        with (
            tc.tile_pool(name="wsb", bufs=1) as wpool,
            tc.tile_pool(name="xt", bufs=xbufs) as xpool,
            tc.tile_pool(name="osb", bufs=obufs) as opool,
            tc.tile_pool(name="psum", bufs=pbufs, space="PSUM") as ppool,
        ):
            w_sb = wpool.tile([128, 8, 6, 128], f32r)
            nc.sync.dma_start(out=w_sb[:], in_=wst_d.rearrange("t c k m -> k t c m"))
            bias_sb = wpool.tile([128, 8], f32)
            nc.sync.dma_start(out=bias_sb[:], in_=bbt_d.rearrange("t k -> k t"))

            def body():
                for nb in range(nblk):
                    n0 = nb * NB
                    xt_sb = xpool.tile([128, M, 2, NB], f32r, name="xt_sb", tag="xt")
                    nc.sync.dma_start(
                        out=xt_sb[:],
                        in_=xt_d[:, :, :, n0 : n0 + NB].rearrange(
                            "j f k n -> k j f n"
                        ),
                    )
                    for half in range(2):
                        pss = [
                            ppool.tile(
                                [128, NB], f32, tag=f"ps{t}", name=f"ps{t}_{nb}"
                            )
                            for t in range(4)
                        ]
                        for tt in range(4):
                            tg = half * 4 + tt
                            i = tg >> 1
                            for cc in range(6):
                                fc = cc & 1
                                j = jl[i][cc >> 1]
                                nc.tensor.matmul(
                                    pss[tt][:],
                                    lhsT=w_sb[:, tg, cc, :],
                                    rhs=xt_sb[:, j, fc, :],
                                    start=(cc == 0),
                                    stop=(cc == 5),
                                )
                        for tt in range(4):
                            tg = half * 4 + tt
                            i, ec = tg >> 1, tg & 1
                            o_sb = opool.tile(
                                [128, NB], f32, name=f"osb{tg}_{nb}", tag="osb"
                            )
                            nc.vector.scalar_tensor_tensor(
                                out=o_sb[:],
                                in0=pss[tt][:],
                                scalar=bias_sb[:, tg : tg + 1],
                                in1=xt_sb[:, i, ec, :].bitcast(f32),
                                op0=add,
                                op1=add,
                            )
                            nc.sync.dma_start(
                                out=out_d[i, ec, :, n0 : n0 + NB], in_=o_sb[:]
                            )

            if repeat > 1:
                with tc.For_i(0, repeat, 1):
                    body()
            else:
                body()
    nc.finalize()
    return nc


def _build_nc_v3(
    nsh=NSH,
    repeat=1,
    xbufs=3,
    obufs=3,
    pbufs=2,
    mode="full",
    stagger=0,
    hints=1,
    dblk=1,
    split=0,
    hiprio=0,
    obf=1,
    pair=0,
    ldwskip=0,
    ccouter=0,
    mmsplit=1,
    unroll=2,
):
    """bf16 out^T formulation with fully-contiguous DMA layouts.

    Host pre-packs x as xt[k, nb, j, fc, n] bf16 so each block's input DMA
    is one 8KB-contiguous chunk per partition (1MB total); the 8 output
    tiles of a block drain into one [128, 8, NB] f32 SBUF tile and leave in
    a single 16KB-per-partition DMA (2MB).  Weights bf16 -> FWL fast
    weight loads; residual "+x[i]" and bias fold into the PSUM-drain STT.
    """
    from concourse import bacc
    import concourse.mybir as mybir
    import concourse.tile as tile

    f32 = mybir.dt.float32
    bf16 = mybir.dt.bfloat16
    NB = 512
    nblk = nsh // NB
    add = mybir.AluOpType.add

    odt = bf16 if obf else f32
    nc = bacc.Bacc(debug=False)
    xt_d = nc.dram_tensor("xt", [128, nblk, M, 2, NB], bf16, kind="ExternalInput")
    wst_d = nc.dram_tensor("wst", [8, 6, 128, 128], bf16, kind="ExternalInput")
    bbt_d = nc.dram_tensor("bbt", [8, 128], f32, kind="ExternalInput")
    out_d = nc.dram_tensor("out", [128, nblk, 8, NB], odt, kind="ExternalOutput")

    jl = [[j for j in range(M) if j != i] for i in range(M)]

    with tile.TileContext(nc) as tc:
        with (
            tc.tile_pool(name="wsb", bufs=1) as wpool,
            tc.tile_pool(name="xt", bufs=xbufs) as xpool,
            tc.tile_pool(name="osb", bufs=obufs) as opool,
            tc.tile_pool(name="psum", bufs=pbufs, space="PSUM") as ppool,
        ):
            w_sb = wpool.tile([128, 8, 6, 128], bf16)
            nc.sync.dma_start(out=w_sb[:], in_=wst_d.rearrange("t c k m -> k t c m"))
            bias_sb = wpool.tile([128, 8], f32)
            nc.sync.dma_start(out=bias_sb[:], in_=bbt_d.rearrange("t k -> k t"))
            if mode == "peonly":
                xt_fix = wpool.tile([128, M, 2, NB], bf16)
                nc.sync.dma_start(out=xt_fix[:], in_=xt_d[:, 0])

            def compute_block(nb, xt_b, o_sb):
                # xt_b: [128, M, 2, NB] view; o_sb: [128, 8, NB] tile
                for half in range(2):
                    pss = [
                        ppool.tile([128, NB], f32, tag=f"ps{t}", name=f"ps{t}_{nb}")
                        for t in range(4)
                    ]
                    ncc = 3 if mode == "halfmm" else 6
                    if ccouter:
                        order = [(tt, cc) for cc in range(ncc) for tt in range(4)]
                    else:
                        order = [(tt, cc) for tt in range(4) for cc in range(ncc)]
                    nbs = NB // mmsplit
                    for tt, cc in order:
                        tg = half * 4 + tt
                        i = tg >> 1
                        fc = cc & 1
                        j = jl[i][cc >> 1]
                        for s in range(mmsplit):
                            nc.tensor.matmul(
                                pss[tt][:, s * nbs : (s + 1) * nbs],
                                lhsT=w_sb[:, tg, cc, :],
                                rhs=xt_b[:, j, fc, s * nbs : (s + 1) * nbs],
                                start=(cc == 0),
                                stop=(cc == ncc - 1),
                            )
                    for tt in range(4):
                        tg = half * 4 + tt
                        i, ec = tg >> 1, tg & 1
                        nc.vector.scalar_tensor_tensor(
                            out=o_sb[:, tg, :],
                            in0=pss[tt][:],
                            scalar=bias_sb[:, tg : tg + 1],
                            in1=xt_b[:, i, ec, :],
                            op0=add,
                            op1=add,
                        )
                    if split and half == 0:
                        nc.scalar.dma_start(
                            out=out_d[:, nb, 0:4], in_=o_sb[:, 0:4, :]
                        )
                if mode == "peonly":
                    nc.scalar.dma_start(out=out_d[:, nb, :, 0:8], in_=o_sb[:, :, 0:8])
                elif split:
                    nc.scalar.dma_start(out=out_d[:, nb, 4:8], in_=o_sb[:, 4:8, :])
                else:
                    nc.scalar.dma_start(out=out_d[:, nb], in_=o_sb[:])

            def compute_pair(nb0, xt_sb, o_sbs):
                # xt_sb: [128, 2, M, 2, NB]; o_sbs: two [128, 8, NB] tiles.
                # Each stationary weight is loaded once and used for both
                # blocks (ldwskip=1 sets ldweights=False on the 2nd matmul).
                for half in range(2):
                    pss = [
                        [
                            ppool.tile(
                                [128, NB],
                                f32,
                                tag=f"ps{t}b{bb}",
                                name=f"ps{t}b{bb}_{nb0}",
                            )
                            for bb in range(2)
                        ]
                        for t in range(4)
                    ]
                    for tt in range(4):
                        tg = half * 4 + tt
                        i = tg >> 1
                        for cc in range(6):
                            fc = cc & 1
                            j = jl[i][cc >> 1]
                            for bb in range(2):
                                mm = nc.tensor.matmul(
                                    pss[tt][bb][:],
                                    lhsT=w_sb[:, tg, cc, :],
                                    rhs=xt_sb[:, bb, j, fc, :],
                                    start=(cc == 0),
                                    stop=(cc == 5),
                                )
                                if ldwskip and bb == 1:
                                    mm.ins.ldweights = False
                    for tt in range(4):
                        tg = half * 4 + tt
                        i, ec = tg >> 1, tg & 1
                        for bb in range(2):
                            nc.vector.scalar_tensor_tensor(
                                out=o_sbs[bb][:, tg, :],
                                in0=pss[tt][bb][:],
                                scalar=bias_sb[:, tg : tg + 1],
                                in1=xt_sb[:, bb, i, ec, :],
                                op0=add,
                                op1=add,
                            )

            def body_pair():
                for nb0 in range(0, nblk, 2):
                    xt_sb = xpool.tile([128, 2, M, 2, NB], bf16, name="xt_sb", tag="xt")
                    nc.sync.dma_start(out=xt_sb[:], in_=xt_d[:, nb0 : nb0 + 2])
                    o_sbs = [
                        opool.tile(
                            [128, 8, NB], odt, name=f"osb_{nb0 + bb}", tag="osb"
                        )
                        for bb in range(2)
                    ]
                    compute_pair(nb0, xt_sb, o_sbs)
                    for bb in range(2):
                        nc.scalar.dma_start(out=out_d[:, nb0 + bb], in_=o_sbs[bb][:])

            def body_peonly():
                for nb in range(nblk):
                    o_sb = opool.tile([128, 8, NB], odt, name=f"osb_{nb}", tag="osb")
                    compute_block(nb, xt_fix, o_sb)

            def body():
                if mode == "peonly":
                    body_peonly()
                    return
                if pair:
                    body_pair()
                    return
                for nb0 in range(0, nblk, dblk):
                    xt_sb = xpool.tile(
                        [128, dblk, M, 2, NB], bf16, name="xt_sb", tag="xt"
                    )
                    if mode != "noin":
                        if hiprio:
                            with tc.high_priority():
                                nc.sync.dma_start(
                                    out=xt_sb[:], in_=xt_d[:, nb0 : nb0 + dblk]
                                )
                        else:
                            nc.sync.dma_start(
                                out=xt_sb[:], in_=xt_d[:, nb0 : nb0 + dblk]
                            )
                    for db in range(dblk):
                        nb = nb0 + db
                        o_sb = opool.tile(
                            [128, 8, NB], odt, name=f"osb_{nb}", tag="osb"
                        )
                        if mode == "dma":
                            nc.gpsimd.memset(o_sb[:], 0.0)
                            nc.scalar.dma_start(out=out_d[:, nb], in_=o_sb[:])
                            continue
                        if mode == "noout":
                            compute_block(nb, xt_sb[:, db], o_sb)  # type: ignore
                            continue
                        compute_block(nb, xt_sb[:, db], o_sb)

            if repeat > 1:
                kw = {}
                if stagger:
                    kw["staggered_reset"] = True
                if hints:
                    kw["hint_engines"] = (mybir.EngineType.PE,)
                if repeat % unroll:
                    unroll = 1
                with tc.For_i(0, repeat // unroll, 1, **kw):
                    for _ in range(unroll):
                        body()
            else:
                body()
    nc.finalize()
    return nc


def _get_exec(**build_kwargs):
    """Build (once per config) the jitted 8-core executor. Returns a callable
    run(xt_g, wst_g, bbt_g, n_iters) -> out_g with global concat arrays."""
    build_kwargs = {"obf": 1, "hints": 1, **build_kwargs}
    key = tuple(sorted(build_kwargs.items()))
    if key in _CACHE:
        return _CACHE[key]

    import jax
    import jax.numpy as jnp
    from jax.sharding import Mesh, PartitionSpec
    from jax.experimental.shard_map import shard_map
    from concourse import bass2jax

    nc = _build_nc_v3(**build_kwargs)
    bass2jax.install_neuronx_cc_hook()

    in_names = ["xt", "wst", "bbt", "out"]
    if nc.partition_id_tensor is not None:
        in_names.append(nc.partition_id_tensor.name)
    out_names = ["out"]
    import ml_dtypes

    out_np_dt = ml_dtypes.bfloat16 if build_kwargs.get("obf") else np.float32
    out_aval = jax.core.ShapedArray((128, NBLK, 8, 512), out_np_dt)

    def _body(xt, wst, bbt, out_zero):
        operands = [xt, wst, bbt, out_zero]
        if nc.partition_id_tensor is not None:
            operands.append(bass2jax.partition_id_tensor())
        outs = bass2jax._bass_exec_p.bind(
            *operands,
            out_avals=(out_aval,),
            in_names=tuple(in_names),
            out_names=tuple(out_names),
            lowering_input_output_aliases=(),
            sim_require_finite=True,
            sim_require_nnan=True,
            nc=nc,
        )
        return tuple(outs)

    devices = jax.devices()[:N_CORES]
    mesh = Mesh(np.asarray(devices), ("core",))
    sharded = jax.jit(
        shard_map(
            _body,
            mesh=mesh,
            in_specs=(PartitionSpec("core"),) * 4,
            out_specs=(PartitionSpec("core"),),
            check_rep=False,
        ),
        keep_unused=True,
    )

    sharding = jax.sharding.NamedSharding(mesh, PartitionSpec("core"))
    _CACHE["sharding"] = sharding
    zeros_fn = jax.jit(
        lambda: jnp.zeros((N_CORES * 128, NBLK, 8, 512), out_np_dt),
        out_shardings=sharding,
    )

    class Exec:
        def call(self, xt_j, wst_j, bbt_j):
            return sharded(xt_j, wst_j, bbt_j, self.out_buf())[0]

        def out_buf(self):
            if not hasattr(self, "_ob"):
                self._ob = zeros_fn()
                import jax as _jax

                _jax.block_until_ready(self._ob)
            return self._ob

        def run(self, xt_g, wst_g, bbt_g, n_iters=1):
            xt_j = jax.device_put(xt_g, sharding)
            wst_j = jax.device_put(wst_g, sharding)
            bbt_j = jax.device_put(bbt_g, sharding)
            outs = None
            for _ in range(n_iters):
                outs = self.call(xt_j, wst_j, bbt_j)
            jax.block_until_ready(outs)
            return outs

    ex = Exec()
    _CACHE[key] = ex
    return ex


def _prep_inputs(x, W, b):
    """Host-side shard + layout prep. Returns global concatenated arrays."""
    import ml_dtypes

    bf16 = ml_dtypes.bfloat16
    x = np.asarray(x, dtype=np.float32)
    W = np.asarray(W, dtype=np.float32)
    b = np.asarray(b, dtype=np.float32)
    n = x.shape[1]
    nsh = n // N_CORES
    nblk = nsh // 512

    # xt_g[c*128 + k, nb, j, fc, n] = x[j, c*nsh + nb*512 + n, fc*128 + k]
    x6 = x.astype(bf16).reshape(M, N_CORES, nblk, 512, 2, 128)
    xt_g = np.ascontiguousarray(x6.transpose(1, 5, 2, 0, 4, 3)).reshape(
        N_CORES * 128, nblk, M, 2, 512
    )

    # Stationary W chunks: wst[(i*2+ec), cc, k, m] = W[i, jl[cc>>1]].T block
    wst = np.empty((8, 6, 128, 128), dtype=np.float32)
    for i in range(M):
        jli = [j for j in range(M) if j != i]
        for ec in range(2):
            t = i * 2 + ec
            for cc in range(6):
                j = jli[cc >> 1]
                fc = cc & 1
                wst[t, cc] = W[i, j][
                    ec * 128 : (ec + 1) * 128, fc * 128 : (fc + 1) * 128
                ].T
    wst_g = np.ascontiguousarray(
        np.broadcast_to(wst.astype(bf16)[None], (N_CORES, 8, 6, 128, 128))
    ).reshape(N_CORES * 8, 6, 128, 128)

    # bias sums: BS[i] = sum_{j != i} b[i, j];  bbt[(i*2+ec), k]
    bs = b.sum(axis=1) - b[np.arange(M), np.arange(M)]  # [4, 256]
    bbt = bs.reshape(8, 128)
    bbt_g = np.ascontiguousarray(
        np.broadcast_to(bbt[None], (N_CORES, 8, 128))
    ).reshape(N_CORES * 8, 128)

    return xt_g, wst_g, bbt_g


def kernel(x, W, b):
    xt_g, wst_g, bbt_g = _prep_inputs(x, W, b)
    ex = _get_exec()
    out_g = ex.run(xt_g, wst_g, bbt_g)
    # out_g: [NC*128, NBLK, 8, 512]; out[i, c*NSH+nb*512+n, ec*128+e]
    #   = out_g[c*128+e, nb, i*2+ec, n]
    og = np.asarray(out_g).reshape(N_CORES, 128, NBLK, M, 2, 512)
    out = np.ascontiguousarray(og.transpose(3, 0, 2, 5, 4, 1))
    if out.dtype != np.float32:
        out = out.astype(np.float32)
    out = out.reshape(M, N, D)
    return out

